# revision 21
# baseline (speedup 1.0000x reference)
"""Multi-head self-attention Trainium2 kernel (8-core head-parallel).

Problem: B=2, N=2048, C=1024, H=16 heads, HD=64.
Sharding: tensor-parallel over heads -- each of the 8 cores computes 2 heads
(QKV slice + attention + partial output projection); the 8 partial projections
are summed on the host (unshard step), along with the projection bias.

All matmuls run as float32r (TF32-like, ~1.6e-4 rel err, full PE rate).
Device-side pipeline per core:
  1. qkv^T = w_loc^T @ x^T   (x^T prepared on host; contraction over C in
     8 chunks of 128), bias added during PSUM->SBUF evacuation (DVE).
  2. v^T re-transposed to natural [token, d] layout on the PE (identity
     matmul), with a constant 1.0 column appended per head so that the
     attn@v matmul also produces the softmax denominators as row 64.
  3. Per (batch, head): scores^T chunks [k=128, q=512] on PE, exp((1/8)s)
     on ACT straight out of PSUM (no max subtraction needed: |s| <~ 8),
     attn@v accumulation over 16 k-chunks into PSUM [65, 512].
  4. Normalization: reciprocal of row 64, partition-broadcast via SWDGE
     replicate DMA, multiply during evacuation (DVE).
  5. Partial projection out_part = oh @ w_proj[rows of this core's heads].
"""

import numpy as np

B, N, C = 2, 2048, 1024
H = 16
HD = C // H  # 64
SCALE = HD ** -0.5
T = B * N  # 4096 tokens
NCORES = 8
HPC = H // NCORES  # 2 heads per core

_CACHE = {}


def _build_program(phases=(1, 2, 3, 4)):
    import concourse.bass as bass
    import concourse.mybir as mybir
    import concourse.tile as tile
    from concourse import bacc

    f32 = mybir.dt.float32
    f32r = mybir.dt.float32r
    Exp = mybir.ActivationFunctionType.Exp
    Mult = mybir.AluOpType.mult

    nc = bacc.Bacc("TRN2", target_bir_lowering=False, debug=False,
                   num_devices=NCORES)

    xT_d = nc.dram_tensor("xT", [C, T], f32, kind="ExternalInput")
    wq_d = nc.dram_tensor("w_loc", [C, 3 * HPC * HD], f32, kind="ExternalInput")
    bq_d = nc.dram_tensor("b_loc", [128, 3], f32, kind="ExternalInput")
    w2_d = nc.dram_tensor("w2_loc", [HPC * HD, C], f32, kind="ExternalInput")
    id_d = nc.dram_tensor("ident", [128, 128], f32, kind="ExternalInput")
    ones_d = nc.dram_tensor("ones2", [128, 2], f32, kind="ExternalInput")
    ones64_d = nc.dram_tensor("ones64", [1, 64], f32, kind="ExternalInput")
    out_d = nc.dram_tensor("out_part", [T, C], f32, kind="ExternalOutput")

    CC = C // 128          # 8 contraction chunks
    NF = 3 * HPC * HD // 128   # 3 feature chunks (q, k, v)
    NTB = T // 512         # 8 token blocks
    NKC = N // 128         # 16 key chunks per batch
    NQB = N // 512         # 4 query blocks per batch
    NTC = T // 128         # 32 token chunks

    with tile.TileContext(nc) as tc:
        with tc.tile_pool(name="persist", bufs=1) as persist, \
             tc.tile_pool(name="xt", bufs=10) as xt_pool, \
             tc.tile_pool(name="exp", bufs=6) as exp_pool, \
             tc.tile_pool(name="small", bufs=4) as small_pool, \
             tc.tile_pool(name="ob", bufs=4) as out_pool, \
             tc.tile_pool(name="ps", bufs=6, space="PSUM") as psum_s, \
             tc.tile_pool(name="po", bufs=2, space="PSUM") as psum_o:

            w_sb = persist.tile([128, CC, 3 * HPC * HD], f32r, tag="w_sb")
            b_sb = persist.tile([128, 3], f32, tag="b_sb")
            w2_sb = persist.tile([128, C], f32r, tag="w2_sb")
            ident = persist.tile([128, 128], f32, tag="ident")
            qT = persist.tile([128, T], f32r, tag="qT")
            kT = persist.tile([128, T], f32r, tag="kT")
            vT = persist.tile([128, T], f32, tag="vT")
            # natural-layout v, per token-chunk: [vA(64) | 1 | vB(64) | 1]
            v_nat = persist.tile([128, NTC, 130], f32r, tag="v_nat")
            ohT = persist.tile([128, T], f32r, tag="ohT")

            # gpsimd DMAs cast f32 -> f32r (rounding in the SDMA datapath)
            nc.gpsimd.dma_start(
                out=w_sb[:],
                in_=wq_d[:].rearrange("(cc p) f -> p cc f", p=128))
            nc.gpsimd.dma_start(out=w2_sb[:], in_=w2_d[:])
            nc.sync.dma_start(out=ident[:], in_=id_d[:])
            nc.sync.dma_start(out=b_sb[:], in_=bq_d[:])
            ones64 = persist.tile([1, 64], f32, tag="ones64")
            nc.sync.dma_start(out=ones64[:], in_=ones64_d[:])

            # ---- phase 1: qkv^T = w_loc^T @ x^T, bias on evacuation ----
            qkvT = [qT, kT, vT]
            for tb in range(NTB) if 1 in phases else ():
                xts = []
                for ci in range(CC):
                    xt = xt_pool.tile([128, 512], f32r)
                    nc.gpsimd.dma_start(
                        out=xt[:],
                        in_=xT_d[ci * 128:(ci + 1) * 128,
                                 tb * 512:(tb + 1) * 512])
                    xts.append(xt)
                for fc in range(NF):
                    ps = psum_s.tile([128, 512], f32, tag="s")
                    for ci in range(CC):
                        nc.tensor.matmul(
                            ps[:],
                            w_sb[:, ci, fc * 128:(fc + 1) * 128],
                            xts[ci][:],
                            start=(ci == 0), stop=(ci == CC - 1))
                    nc.vector.tensor_scalar_add(
                        qkvT[fc][:, tb * 512:(tb + 1) * 512],
                        ps[:], b_sb[:, fc:fc + 1])

            # ---- phase 1.5: v^T -> natural layout (PE transpose) ----
            for tcg in range(NTC) if 2 in phases else ():
                pt = psum_o.tile([128, 128], f32, tag="po")
                sl = slice(tcg * 128, (tcg + 1) * 128)
                # single full-array transpose of both heads' v^T chunk
                nc.tensor.transpose(pt[:], vT[:, sl], ident[:])
                nc.vector.tensor_copy(v_nat[:, tcg, 0:64], pt[:, 0:64])
                nc.vector.tensor_copy(v_nat[:, tcg, 65:129], pt[:, 64:128])
            # constant 1.0 columns (per-head softmax-denominator rows),
            # broadcast over token chunks from a tiny host input
            ones_ap = ones_d[:]
            for col, off in ((64, 0), (129, 1)) if 2 in phases else ():
                nc.gpsimd.dma_start(
                    out=v_nat[:, :, col:col + 1],
                    in_=bass.AP(tensor=ones_ap.tensor, offset=off,
                                ap=[[2, 128], [0, NTC], [1, 1]]))

            # ---- phase 2: attention per (batch, head) ----
            for b in range(B) if 3 in phases else ():
                for qb in range(NQB):
                    qsl = slice(b * N + qb * 512, b * N + (qb + 1) * 512)
                    po = [psum_o.tile([128, 512], f32, tag="po",
                                      name=f"po_{b}_{qb}_{h}")
                          for h in range(HPC)]
                    for kcg in range(NKC // 2):
                        exs = {}
                        for kc2 in range(2):
                            kc = kcg * 2 + kc2
                            ksl = slice(b * N + kc * 128,
                                        b * N + (kc + 1) * 128)
                            for h in range(HPC):
                                hsl = slice(h * 64, (h + 1) * 64)
                                ps = psum_s.tile([128, 512], f32, tag="s")
                                nc.tensor.matmul(
                                    ps[:], kT[hsl, ksl], qT[hsl, qsl],
                                    start=True, stop=True)
                                ex = exp_pool.tile([128, 512], f32r)
                                nc.scalar.activation(ex[:], ps[:], Exp,
                                                     scale=float(SCALE))
                                exs[(kc2, h)] = ex
                        for kc2 in range(2):
                            kc = kcg * 2 + kc2
                            tcg = b * NKC + kc
                            for h in range(HPC):
                                nc.tensor.matmul(
                                    po[h][0:65, :],
                                    v_nat[:, tcg, h * 65:(h + 1) * 65],
                                    exs[(kc2, h)][:],
                                    start=(kc == 0), stop=(kc == NKC - 1))
                    for h in range(HPC):
                        # broadcast sums row across partitions via a PE
                        # outer product (ones column x sums row), then
                        # reciprocal + multiply on DVE
                        s_sb = small_pool.tile([1, 512], f32, tag="r")
                        nc.vector.tensor_copy(s_sb[:], po[h][64:65, :])
                        pr = psum_s.tile([64, 512], f32, tag="s")
                        nc.tensor.matmul(pr[:], ones64[:], s_sb[:],
                                         start=True, stop=True)
                        rcp = small_pool.tile([64, 512], f32, tag="rb")
                        nc.vector.reciprocal(rcp[:], pr[:])
                        nc.vector.tensor_tensor(
                            ohT[h * 64:(h + 1) * 64, qsl],
                            po[h][0:64, :], rcp[:], Mult)

            # ---- phase 3: partial projection ----
            for tcg in range(NTC) if 4 in phases else ():
                for jh in range(C // 512):
                    pp = psum_s.tile([128, 512], f32, tag="s")
                    nc.tensor.matmul(
                        pp[:], ohT[:, tcg * 128:(tcg + 1) * 128],
                        w2_sb[:, jh * 512:(jh + 1) * 512],
                        start=True, stop=True)
                    ob = out_pool.tile([128, 512], f32)
                    nc.vector.tensor_copy(ob[:], pp[:])
                    nc.sync.dma_start(
                        out=out_d[tcg * 128:(tcg + 1) * 128,
                                  jh * 512:(jh + 1) * 512],
                        in_=ob[:])

    nc.compile()
    return nc


def get_program():
    if "nc" not in _CACHE:
        _CACHE["nc"] = _build_program()
    return _CACHE["nc"]


def build_null_program():
    """Tiny kernel for calibrating per-dispatch overhead in test harnesses."""
    import concourse.mybir as mybir
    import concourse.tile as tile
    from concourse import bacc

    f32 = mybir.dt.float32
    nc = bacc.Bacc("TRN2", target_bir_lowering=False, debug=False,
                   num_devices=NCORES)
    x_in = nc.dram_tensor("x", [128, 128], f32, kind="ExternalInput")
    y_out = nc.dram_tensor("y", [128, 128], f32, kind="ExternalOutput")
    with tile.TileContext(nc) as tc:
        with tc.tile_pool(name="p", bufs=1) as pool:
            t = pool.tile([128, 128], f32)
            nc.sync.dma_start(out=t[:], in_=x_in[:])
            nc.sync.dma_start(out=y_out[:], in_=t[:])
    nc.compile()
    x = np.zeros((128, 128), dtype=np.float32)
    return nc, [{"x": x} for _ in range(NCORES)]


def make_in_maps(x, w_qkv, b_qkv, w_proj):
    """Host-side sharding: per-core input dicts."""
    xT = np.ascontiguousarray(x.reshape(T, C).T).astype(np.float32)
    ident = np.eye(128, dtype=np.float32)
    in_maps = []
    for core in range(NCORES):
        heads = [core * HPC + h for h in range(HPC)]
        # qkv feature columns for this core, ordered [qA qB kA kB vA vB]
        cols = []
        for s in range(3):  # q, k, v groups
            for h in heads:
                cols.append(np.arange(s * C + h * HD, s * C + (h + 1) * HD))
        cols = np.concatenate(cols)
        w_loc = np.ascontiguousarray(w_qkv[:, cols]).astype(np.float32)
        b_loc = np.ascontiguousarray(
            b_qkv[cols].reshape(3, HPC * HD).T).astype(np.float32)
        rows = np.concatenate(
            [np.arange(h * HD, (h + 1) * HD) for h in heads])
        w2_loc = np.ascontiguousarray(w_proj[rows, :]).astype(np.float32)
        in_maps.append({
            "xT": xT,
            "w_loc": w_loc,
            "b_loc": b_loc,
            "w2_loc": w2_loc,
            "ident": ident,
            "ones2": np.ones((128, 2), dtype=np.float32),
            "ones64": np.ones((1, 64), dtype=np.float32),
        })
    return in_maps


def combine_results(results, b_proj):
    """Host-side unshard: sum the 8 partial projections, add bias."""
    acc = np.zeros((T, C), dtype=np.float32)
    for res in results:
        acc += res["out_part"]
    acc += b_proj.astype(np.float32)[None, :]
    return acc.reshape(B, N, C)


def kernel(x, w_qkv, b_qkv, w_proj, b_proj):
    from concourse.bass_utils import run_bass_kernel_spmd

    x = np.asarray(x, dtype=np.float32)
    w_qkv = np.asarray(w_qkv, dtype=np.float32)
    b_qkv = np.asarray(b_qkv, dtype=np.float32)
    w_proj = np.asarray(w_proj, dtype=np.float32)
    b_proj = np.asarray(b_proj, dtype=np.float32)

    nc = get_program()
    in_maps = make_in_maps(x, w_qkv, b_qkv, w_proj)
    res = run_bass_kernel_spmd(nc, in_maps, list(range(NCORES)))
    return combine_results(res.results, b_proj)
